# revision 4
# baseline (speedup 1.0000x reference)
"""Trainium2 Bass kernel for nn_DisLoss: loss = sum(x * dist_to_argmax(x)) / b.

x: (128, 512, 512) f32. Data-parallel over 8 NeuronCores: 16 images per core.

Distance-field trick: dist(r,c) = T[r-cy+512, c-cx+512] where T is the fixed
1024x1024 table sqrt((i-512)^2+(j-512)^2). T is symmetric with numerical
rank ~16: T = sum_k sigma_k w_k w_k^T (eigendecomposition). So

  S_img = sum_{r,c} x[r,c] dist(r,c)
        = sum_k sigma_k (sum_r w_k(r-cy+512) x[r,c]) w_k(c-cx+512)
        = sum_k sigma_k sum_c y[k,c] w_k(c-cx+512)

Per image on device:
  1. one DMA (pairs of images share one 2MB DMA) + one DVE reduce_max over
     all rows -> rowmax; exact fp32 argmax located via the encoded-max trick
     (Pool does the cross-partition reductions with C-axis tensor_reduce).
  2. indirect-DMA gather of W rows (shifted by 512-cy, via the 4-row
     redundant WQUAD table) -> lhsT [128,4x16] in float32r.
  3. 4 PE fp32r matmuls contract rows: y[k,c] = sum_r W[r-cy+512,k] x[r,c].
  4. ACT copies y PSUM->SBUF; y ships to DRAM along with the (cy,cx)
     encodings. The host applies the V-side contraction in float64.
Engines all sit below the ~350 GB/s per-core DMA roofline (~2.9us/image).
"""

import numpy as np

B_FULL = 128
H = 512
W = 512
N_CORES = 8
B_CORE = B_FULL // N_CORES  # 16 images per core
T = 4                       # rows per partition
K = 16                      # factor rank
ROWP = 32                   # partitions per image for the gathered-row scan
ROWF = 16                   # free elems per partition in the row scan
NPAIR = B_CORE // 2

_CACHE = {}


def _factors():
    i = np.arange(1024, dtype=np.float64) - 512.0
    Tm = np.sqrt(i[:, None] ** 2 + i[None, :] ** 2)
    lam, Q = np.linalg.eigh(Tm)
    idx = np.argsort(-np.abs(lam))[:K]
    lam_k = lam[idx]
    Wh = Q[:, idx] * np.sqrt(np.abs(lam_k))[None, :]   # [1024, K] float64
    sigma = np.sign(lam_k)                              # [K]
    return Wh, sigma


def _consts():
    Wh, _sigma = _factors()
    Wf = Wh.astype(np.float32)                          # [1024, K]
    wq = np.zeros((1024, 4 * K), dtype=np.float32)
    for a in range(4):
        hi = 1024 - a
        wq[:hi, a * K:(a + 1) * K] = Wf[a:, :]
    r4 = (4.0 * np.arange(128, dtype=np.float32)[:, None]
          + np.arange(T, dtype=np.float32)[None, :])
    r4enc8 = (32.0 * (4096.0 - r4)).copy()              # [128, 4]
    pidx64 = (np.arange(64, dtype=np.float32)[:, None] % 32).copy()
    lin = (ROWF * (np.arange(64, dtype=np.float32)[:, None] % 32)
           + np.arange(ROWF, dtype=np.float32)[None, :])
    colenc64 = (512.0 - lin).astype(np.float32)         # [64, 16]
    pidx4 = (4.0 * np.arange(128, dtype=np.float32)[:, None]).copy()
    ones = np.ones((1, 128), dtype=np.float32)
    return {"WQUAD": wq, "R4ENC8": r4enc8, "PIDX64": pidx64,
            "COLENC64": colenc64, "PIDX4": pidx4, "ONES": ones}


def build_program(debug=False, hwloop=0, stages_upto=None):
    import concourse.bass as bass
    import concourse.bass_isa as bass_isa
    import concourse.bacc as bacc
    import concourse.mybir as mybir
    from concourse.tile import TileContext

    nb = B_CORE
    f32 = mybir.dt.float32
    f32r = mybir.dt.float32r
    u32 = mybir.dt.uint32
    Alu = mybir.AluOpType
    Act = mybir.ActivationFunctionType
    Ax = mybir.AxisListType

    nc = bacc.Bacc("TRN2", target_bir_lowering=False, debug=False)

    x_d = nc.dram_tensor("x", [nb, H, W], f32r, kind="ExternalInput")
    wq_d = nc.dram_tensor("WQUAD", [1024, 4 * K], f32r, kind="ExternalInput")
    r4enc8_d = nc.dram_tensor("R4ENC8", [128, T], f32, kind="ExternalInput")
    pidx64_d = nc.dram_tensor("PIDX64", [64, 1], f32, kind="ExternalInput")
    colenc64_d = nc.dram_tensor("COLENC64", [64, ROWF], f32,
                                kind="ExternalInput")
    pidx4_d = nc.dram_tensor("PIDX4", [128, 1], f32, kind="ExternalInput")
    ones_d = nc.dram_tensor("ONES", [1, 128], f32, kind="ExternalInput")
    ys_d = nc.dram_tensor("ys", [K, nb * W], f32, kind="ExternalOutput")
    locs_d = nc.dram_tensor("locs", [1, 2 * nb], f32, kind="ExternalOutput")

    x_ap = x_d.ap()
    x_rows = x_ap.rearrange("b h (s i) -> (b h s) i", i=ROWF)
    wq_rows = wq_d.ap()

    with TileContext(nc) as tc:
        with (
            tc.tile_pool(name="consts", bufs=1) as consts,
            tc.tile_pool(name="xs", bufs=8) as xs,
            tc.tile_pool(name="small", bufs=10) as small,
            tc.tile_pool(name="rows", bufs=8) as rows,
            tc.tile_pool(name="psy", bufs=6, space="PSUM") as psy,
        ):
            r4enc8_t = consts.tile([128, T], f32)
            nc.sync.dma_start(out=r4enc8_t, in_=r4enc8_d.ap())
            pidx64_t = consts.tile([64, 1], f32)
            nc.sync.dma_start(out=pidx64_t, in_=pidx64_d.ap())
            colenc64_t = consts.tile([64, ROWF], f32)
            nc.sync.dma_start(out=colenc64_t, in_=colenc64_d.ap())
            pidx4_t = consts.tile([128, 1], f32)
            nc.sync.dma_start(out=pidx4_t, in_=pidx4_d.ap())
            ones_t = consts.tile([1, 128], f32)
            nc.sync.dma_start(out=ones_t, in_=ones_d.ap())

            ys_all = consts.tile([K, nb * W], f32)
            locs_t = consts.tile([1, 2 * nb], f32)
            m4096_t = consts.tile([128, 1], f32)
            nc.vector.memset(m4096_t, -4096.0)
            if stages_upto is not None:
                nc.vector.memset(ys_all, 0.0)
                nc.vector.memset(locs_t, 0.0)

            def body():
                def s0(st):
                    u = st["u"]
                    x_t = xs.tile([128, 2, T, W], f32r)
                    nc.sync.dma_start(
                        out=x_t,
                        in_=x_ap[2 * u:2 * u + 2].rearrange(
                            "b (p t) w -> p b t w", p=128))
                    rowmax = small.tile([128, 2, T], f32)
                    nc.vector.tensor_reduce(
                        rowmax, x_t[:, :, :, :].bitcast(f32),
                        axis=Ax.X, op=Alu.max)
                    pmaxs = []
                    for s in range(2):
                        pm = small.tile([128, 1], f32, tag=f"pm{s}")
                        nc.vector.reduce_max(pm, rowmax[:, s, :], axis=Ax.X)
                        pmaxs.append(pm)
                    st.update(x_t=x_t, rowmax=rowmax, pmaxs=pmaxs)

                def s1(st):
                    msbs = []
                    for s in range(2):
                        msb = small.tile([128, 1], f32, tag=f"msb{s}")
                        nc.gpsimd.partition_all_reduce(
                            msb, st["pmaxs"][s], channels=128,
                            reduce_op=bass_isa.ReduceOp.max)
                        msbs.append(msb)
                    st.update(msbs=msbs)

                def s2(st):
                    u = st["u"]
                    e1bs = []
                    for s in range(2):
                        b = 2 * u + s
                        e4 = small.tile([128, T], f32, tag=f"e4{s}")
                        junk = small.tile([128, 1], f32, tag=f"j4{s}")
                        nc.vector.scalar_tensor_tensor(
                            e4, st["rowmax"][:, s, :],
                            st["msbs"][s], r4enc8_t,
                            op0=Alu.is_equal, op1=Alu.mult, accum_out=junk)
                        ep = small.tile([128, 1], f32, tag=f"ep{s}")
                        nc.vector.reduce_max(ep, e4, axis=Ax.X)
                        e1b = small.tile([128, 1], f32, tag=f"e1b{s}")
                        nc.gpsimd.partition_all_reduce(
                            e1b, ep, channels=128,
                            reduce_op=bass_isa.ReduceOp.max)
                        nc.vector.tensor_copy(
                            locs_t[0:1, 2 * b:2 * b + 1], e1b[0:1, 0:1])
                        e1bs.append(e1b)
                    st.update(e1bs=e1bs)

                def s3(st):
                    u = st["u"]
                    cyu = small.tile([64, 1], u32, tag="cyu")
                    woffs = []
                    for s in range(2):
                        b = 2 * u + s
                        e1b = st["e1bs"][s]
                        negcy = small.tile([128, 1], f32, tag=f"ncy{s}")
                        nc.scalar.activation(
                            negcy, e1b, Act.Identity,
                            scale=1.0 / 32.0, bias=m4096_t)
                        nc.vector.tensor_scalar(
                            cyu[32 * s:32 * s + 32, 0:1],
                            pidx64_t[32 * s:32 * s + 32, 0:1],
                            e1b[32 * s:32 * s + 32, 0:1],
                            float(32 * 4096 + ROWP * H * b),
                            op0=Alu.subtract, op1=Alu.add)
                        woff = small.tile([128, 1], u32, tag=f"woff{s}")
                        nc.vector.tensor_scalar(
                            woff, pidx4_t, negcy, 512.0,
                            op0=Alu.add, op1=Alu.add)
                        woffs.append(woff)
                    st.update(cyu=cyu, woffs=woffs)

                def s4(st):
                    rowbuf = rows.tile([64, ROWF], f32r, tag="rowbuf")
                    nc.gpsimd.indirect_dma_start(
                        out=rowbuf, out_offset=None,
                        in_=x_rows,
                        in_offset=bass.IndirectOffsetOnAxis(
                            ap=st["cyu"][:], axis=0))
                    wqts = []
                    for s in range(2):
                        wqt = rows.tile([128, 4 * K], f32r, tag=f"wq{s}")
                        nc.gpsimd.indirect_dma_start(
                            out=wqt, out_offset=None,
                            in_=wq_rows,
                            in_offset=bass.IndirectOffsetOnAxis(
                                ap=st["woffs"][s][:], axis=0))
                        wqts.append(wqt)
                    st.update(rowbuf=rowbuf, wqts=wqts)

                def s5(st):
                    u = st["u"]
                    x_t, wqts = st["x_t"], st["wqts"]
                    yps = []
                    for s in range(2):
                        b = 2 * u + s
                        if stages_upto != 66:
                            ohr = small.tile([32, ROWF], f32, tag=f"ohr{s}")
                            junk = small.tile([32, 1], f32, tag=f"jr{s}")
                            nc.vector.scalar_tensor_tensor(
                                ohr,
                                st["rowbuf"][32 * s:32 * s + 32, :].bitcast(f32),
                                st["msbs"][s][32 * s:32 * s + 32, 0:1],
                                colenc64_t[32 * s:32 * s + 32, :],
                                op0=Alu.is_equal, op1=Alu.mult, accum_out=junk)
                            ecp = small.tile([32, 1], f32, tag=f"ecp{s}")
                            nc.vector.reduce_max(ecp, ohr, axis=Ax.X)
                            e2b = small.tile([32, 1], f32, tag=f"e2b{s}")
                            nc.gpsimd.partition_all_reduce(
                                e2b, ecp, channels=32,
                                reduce_op=bass_isa.ReduceOp.max)
                            nc.vector.tensor_copy(
                                locs_t[0:1, 2 * b + 1:2 * b + 2], e2b[0:1, 0:1])
                        y_ps = psy.tile([K, W], f32, tag="y")
                        for t in range(T):
                            nc.tensor.matmul(
                                y_ps, wqts[s][:, t * K:(t + 1) * K],
                                x_t[:, s, t, :],
                                start=(t == 0), stop=(t == T - 1))
                        yps.append(y_ps)
                    st.update(yps=yps)

                def s6(st):
                    u = st["u"]
                    for s in range(2):
                        b = 2 * u + s
                        nc.scalar.copy(
                            ys_all[:, b * W:(b + 1) * W], st["yps"][s])

                stages = [s0, s1, s2, s3, s4, s5]
                if stages_upto is not None:
                    stages = stages[:min(stages_upto, 6)]
                n = NPAIR
                states = {}
                NS = len(stages) + 1
                for i in range(n + NS - 1):
                    j = i - (NS - 1)
                    if 0 <= j < n:
                        if stages_upto is None:
                            s6(states.pop(j))
                        else:
                            states.pop(j)
                    for k in range(NS - 2, -1, -1):
                        j = i - k
                        if 0 <= j < n:
                            if k == 0:
                                states[j] = {"u": j}
                            stages[k](states[j])

            if hwloop:
                with tc.For_i(0, hwloop):
                    body()
            else:
                body()

            nc.sync.dma_start(out=ys_d.ap(), in_=ys_all)
            nc.sync.dma_start(out=locs_d.ap(), in_=locs_t)

    nc.compile()
    return nc


def _host_reduce(results):
    """Apply the V-side contraction in float64 on the host."""
    Wh, sigma = _factors()
    Ws = Wh * sigma[None, :]                      # [1024, K] float64
    total = 0.0
    cidx = np.arange(W)
    for r in results:
        ys = r["ys"].astype(np.float64)           # [K, nb*W]
        locs = r["locs"][0]                       # [2*nb]
        for b in range(B_CORE):
            e1 = locs[2 * b]
            e2 = locs[2 * b + 1]
            cy = int(round(4096.0 - e1 / 32.0))
            cx = int(round(512.0 - e2))
            y = ys[:, b * W:(b + 1) * W]          # [K, 512]
            Vs = Ws[cidx - cx + 512, :]           # [512, K]
            total += float(np.einsum("kc,ck->", y, Vs))
    return total


def kernel(x: np.ndarray) -> np.ndarray:
    from concourse import bass_utils

    key = "nc"
    if key not in _CACHE:
        _CACHE[key] = build_program(debug=False)
    nc = _CACHE[key]

    x = np.ascontiguousarray(x, dtype=np.float32)
    shards = x.reshape(N_CORES, B_CORE, H, W)
    consts = _consts()
    in_maps = [dict(consts, x=shards[i]) for i in range(N_CORES)]
    res = bass_utils.run_bass_kernel_spmd(
        nc, in_maps, core_ids=list(range(N_CORES)))
    total = _host_reduce(res.results)
    return np.float32(total / B_FULL)
